# revision 19
# baseline (speedup 1.0000x reference)
"""Trainium2 Bass kernel for nn_MultiHeadSelfAttention_72748156059791.

Reference computation (B=4, S=1024, E=1024, N=16 heads, D=64):
    Q = X @ WQ[n];  K = X @ WK[n]
    scores = (K @ Q^T) / 8, upper-triangular kept, rest -> -inf
    attn = softmax(scores, axis=-1)
    v_down = diag(attn) * (X @ WV_down[n])     # only the DIAGONAL of attn is used!
    V = v_down @ WV_up[n]
    out = X + mean_n(V)

Key algorithmic facts exploited:
  * attn enters only through its diagonal -> no attn @ V matmul; we only need
    per-row softmax stats: Z[s] = sum_{t>=s} exp(scores[s,t]) and exp(scores[s,s]).
  * mean over heads is a linear op -> folded into WV_up (pre-scaled by 1/16)
    and the per-head V contributions are summed by PSUM accumulation using
    d-stacking: [vd_a | vd_b] @ [Wup_a ; Wup_b] = V_a + V_b.

Sharding: 8 cores = 4 batches x 2 head-groups (8 heads each). Each core
returns a [1024,1024] bf16 partial delta (sum of its 8 heads' V, already
/16); host adds the two partials per batch to the input.

All matmuls run in bf16 (fp32 PSUM accumulation); validated numerics:
scale-relative absmax error ~7e-5 vs the fp32 reference.
"""

import sys

import numpy as np
import ml_dtypes

if "/opt/trn_rl_repo" not in sys.path:
    sys.path.insert(0, "/opt/trn_rl_repo")

BF16 = ml_dtypes.bfloat16

B, S, E = 4, 1024, 1024
N, D = 16, 64
NCORES = 8
NPG = 4  # head pair-groups per core (2 heads each -> 8 heads/core)

_CACHE = {}


def _build():
    import concourse.bacc as bacc
    import concourse.tile as tile
    import concourse.mybir as mybir

    f32 = mybir.dt.float32
    bf16 = mybir.dt.bfloat16
    EXP = mybir.ActivationFunctionType.Exp

    nc = bacc.Bacc("TRN2", target_bir_lowering=False)

    xt_d = nc.dram_tensor("xt", [128, 8, 1024], bf16, kind="ExternalInput")
    wq_d = nc.dram_tensor("wq", [128, NPG, 8, 128], bf16, kind="ExternalInput")
    wk_d = nc.dram_tensor("wk", [128, NPG, 8, 128], bf16, kind="ExternalInput")
    wvd_d = nc.dram_tensor("wvd", [128, NPG, 8, 128], bf16, kind="ExternalInput")
    wvu_d = nc.dram_tensor("wvu", [128, NPG, 1024], bf16, kind="ExternalInput")
    iden_d = nc.dram_tensor("iden", [128, 128], bf16, kind="ExternalInput")
    trilm_d = nc.dram_tensor("trilm", [128, 128], f32, kind="ExternalInput")
    sel2_d = nc.dram_tensor("sel2", [128, 2], bf16, kind="ExternalInput")
    id2_d = nc.dram_tensor("id2", [2, 2], f32, kind="ExternalInput")
    out_d = nc.dram_tensor("out", [8, 128, 1024], bf16, kind="ExternalOutput")

    with tile.TileContext(nc) as tc:
        with (
            tc.tile_pool(name="cst", bufs=1) as cst,
            tc.tile_pool(name="wx", bufs=1) as wx,
            tc.tile_pool(name="proj", bufs=2) as proj,
            tc.tile_pool(name="vd4", bufs=4) as vd4,
            tc.tile_pool(name="st", bufs=2) as stp,
            tc.tile_pool(name="scr", bufs=1) as scrp,
            tc.tile_pool(name="ob", bufs=4) as obp,
            tc.tile_pool(name="ps_strip", bufs=2, space="PSUM") as ps_strip,
            tc.tile_pool(name="ps_proj", bufs=2, space="PSUM") as ps_proj,
            tc.tile_pool(name="ps_acc", bufs=2, space="PSUM") as ps_acc,
        ):
            # ---- constants & weights (loaded once) ----
            iden = cst.tile([128, 128], bf16, tag="iden")
            trilm = cst.tile([128, 128], f32, tag="trilm")
            sel2 = cst.tile([128, 2], bf16, tag="sel2")
            id2 = cst.tile([2, 2], f32, tag="id2")
            nc.sync.dma_start(iden[:], iden_d[:])
            nc.sync.dma_start(trilm[:], trilm_d[:])
            nc.sync.dma_start(sel2[:], sel2_d[:])
            nc.sync.dma_start(id2[:], id2_d[:])

            xt = wx.tile([128, 8, 1024], bf16, tag="xt")
            wq = wx.tile([128, NPG, 8, 128], bf16, tag="wq")
            wk = wx.tile([128, NPG, 8, 128], bf16, tag="wk")
            wvd = wx.tile([128, NPG, 8, 128], bf16, tag="wvd")
            wvu = wx.tile([128, NPG, 1024], bf16, tag="wvu")
            # pg0 weights + x chunks first so projections start ASAP
            nc.sync.dma_start(wq[:, 0, :, :], wq_d[:, 0, :, :])
            for ch in range(8):
                nc.sync.dma_start(xt[:, ch, 0:512], xt_d[:, ch, 0:512])
            for ch in range(8):
                nc.sync.dma_start(xt[:, ch, 512:1024], xt_d[:, ch, 512:1024])
            nc.sync.dma_start(wk[:, 0, :, :], wk_d[:, 0, :, :])
            nc.sync.dma_start(wvd[:, 0, :, :], wvd_d[:, 0, :, :])
            for pg in range(1, NPG):
                for t, d in ((wq, wq_d), (wk, wk_d), (wvd, wvd_d)):
                    nc.sync.dma_start(t[:, pg, :, :], d[:, pg, :, :])
            for pg in range(NPG):
                nc.sync.dma_start(wvu[:, pg, :], wvu_d[:, pg, :])

            vdst = [vd4.tile([128, 1024], bf16, tag="vdst", name=f"vdst{_pg}") for _pg in range(NPG)]

            # per-pg projection targets; groups are emitted lazily so that the
            # next pg's projection matmuls interleave with (ACT-bound) scores
            # strips of the current pg, keeping TensorE dense (HAM warm).
            projs = {}

            def alloc_proj(pg):
                q_sb = proj.tile([128, 1024], bf16, tag="q", name=f"q{pg}")
                k_sb = proj.tile([128, 1024], bf16, tag="k", name=f"k{pg}")
                vd_sb = proj.tile([128, 1024], bf16, tag="vd", name=f"vd{pg}")
                projs[pg] = (q_sb, k_sb, vd_sb)

            def emit_proj_group(pg, which, sthalf):
                wsb = (wq, wk, wvd)[which]
                dst = projs[pg][which]
                ps = ps_proj.tile(
                    [128, 512], f32, tag="pp", name=f"pp_{pg}_{which}_{sthalf}"
                )
                cols = slice(512 * sthalf, 512 * (sthalf + 1))
                for ch in range(8):
                    nc.tensor.matmul(
                        ps[:],
                        wsb[:, pg, ch, :],
                        xt[:, ch, cols],
                        start=(ch == 0),
                        stop=(ch == 7),
                    )
                nc.vector.tensor_copy(dst[:, cols], ps[:])

            alloc_proj(0)
            for which in range(3):
                for sthalf in range(2):
                    emit_proj_group(0, which, sthalf)

            def emit_bcast_vdst(pg, sthalf, reps, vd_sb):
                bc = ps_acc.tile(
                    [128, 512], f32, tag="acc", name=f"bc{pg}_{sthalf}"
                )
                for il in range(4):
                    i = 4 * sthalf + il
                    for h2 in range(2):
                        nc.tensor.matmul(
                            bc[64 * h2 : 64 * (h2 + 1), 128 * il : 128 * (il + 1)],
                            reps[h2][:, i : i + 1].broadcast_to((128, 64)),
                            iden[:],
                            start=True,
                            stop=True,
                            tile_position=(0, 64 * h2),
                        )
                cols = slice(512 * sthalf, 512 * (sthalf + 1))
                nc.vector.tensor_mul(vdst[pg][:, cols], vd_sb[:, cols], bc)

            for pg in range(NPG):
                q_sb, k_sb, vd_sb = projs[pg]
                pending = []
                if pg + 1 < NPG:
                    alloc_proj(pg + 1)
                    pending = [
                        (lambda a=pg + 1, w=w, s=s: emit_proj_group(a, w, s))
                        for w in range(3)
                        for s in range(2)
                    ]

                # ---- scores row-strips + unmasked exp (accumulates Za incl.
                # sub-diagonal junk of the diag block; corrected below) ----
                zas = [stp.tile([128, 8], f32, tag=f"za{h2}", name=f"za{h2}") for h2 in range(2)]
                scr = [
                    scrp.tile([128, 8, 1024], f32, tag=f"scr{h2}", name=f"scr{h2}")
                    for h2 in range(2)
                ]
                tmpms = [
                    stp.tile([128, 8, 128], f32, tag=f"tmpm{h2}", name=f"tmpm{h2}")
                    for h2 in range(2)
                ]
                zcs = [
                    stp.tile([128, 8], f32, tag=f"zc{h2}", name=f"zc{h2}")
                    for h2 in range(2)
                ]
                for i in range(8):
                    c0 = 128 * i
                    strips = []
                    for h2 in range(2):
                        pss = ps_strip.tile([128, 1024], f32, tag="strip")
                        strips.append(pss)
                        hp = slice(64 * h2, 64 * (h2 + 1))
                        segs = [(c0, 512), (512, 1024)] if c0 < 512 else [(c0, 1024)]
                        for lo, hi in segs:
                            nc.tensor.matmul(
                                pss[:, lo:hi],
                                k_sb[hp, c0 : c0 + 128],
                                q_sb[hp, lo:hi],
                                start=True,
                                stop=True,
                                tile_position=(64 * h2, 0),
                            )
                    for h2 in range(2):
                        nc.scalar.activation(
                            scr[h2][:, i, 0 : 1024 - c0],
                            strips[h2][:, c0:1024],
                            EXP,
                            accum_out=zas[h2][:, i : i + 1],
                        )
                    if pending:
                        pending.pop(0)()
                    if i == 3:
                        for h2 in range(2):
                            nc.vector.tensor_mul(
                                tmpms[h2][:, 0:4, :],
                                scr[h2][:, 0:4, 0:128],
                                trilm[:].unsqueeze(1).broadcast_to((128, 4, 128)),
                            )
                            nc.vector.tensor_reduce(
                                zcs[h2][:, 0:4],
                                tmpms[h2][:, 0:4, :],
                                axis=mybir.AxisListType.X,
                                op=mybir.AluOpType.add,
                            )
                while pending:
                    pending.pop(0)()

                # ---- diagonal of scores: d[s] = K[s].Q[s], via K^T*Q^T + selector matmul ----
                p_sb = proj.tile([128, 1024], bf16, tag="p")
                nc.vector.tensor_mul(p_sb[:], q_sb[:], k_sb[:])
                edt = stp.tile([2, 1024], f32, tag="edt")
                dt_ps = ps_strip.tile([2, 1024], f32, tag="strip")
                for sthalf in range(2):
                    cols = slice(512 * sthalf, 512 * (sthalf + 1))
                    nc.tensor.matmul(
                        dt_ps[:, cols], sel2[:], p_sb[:, cols], start=True, stop=True
                    )
                nc.scalar.activation(edt[:], dt_ps[:], EXP)
                # transpose exp(diag) back to [s-partition] layout, then evacuate
                # to SBUF immediately to free the PSUM slot
                ed_ps = ps_proj.tile([128, 2, 8], f32, tag="pp", name=f"edps{pg}")
                for i in range(8):
                    nc.tensor.transpose(
                        ed_ps[:, :, i], edt[:, 128 * i : 128 * (i + 1)], id2[:]
                    )
                ed_sb = stp.tile([128, 2, 8], f32, tag="ed_sb")
                nc.vector.tensor_copy(ed_sb[:], ed_ps[:])

                # ---- attn diag: Z = Za - (sub-diagonal junk sums), batched ----
                reps = []
                for h2 in range(2):
                    nc.vector.tensor_mul(
                        tmpms[h2][:, 4:8, :],
                        scr[h2][:, 4:8, 0:128],
                        trilm[:].unsqueeze(1).broadcast_to((128, 4, 128)),
                    )
                    nc.vector.tensor_reduce(
                        zcs[h2][:, 4:8],
                        tmpms[h2][:, 4:8, :],
                        axis=mybir.AxisListType.X,
                        op=mybir.AluOpType.add,
                    )
                    z = stp.tile([128, 8], f32, tag=f"z{h2}")
                    nc.vector.tensor_sub(z[:], zas[h2][:], zcs[h2][:])
                    r = stp.tile([128, 8], f32, tag=f"r{h2}")
                    nc.vector.reciprocal(r[:], z[:])
                    at = stp.tile([128, 8], bf16, tag=f"at{h2}")
                    nc.vector.tensor_mul(at[:], r[:], ed_sb[:, h2, :])
                    reps.append(at)
                for sthalf in range(2):
                    emit_bcast_vdst(pg, sthalf, reps, vd_sb)

            # ---- V up-projection: accumulate all 8 heads (4 pg) in PSUM ----
            for sb in range(8):
                for et in range(2):
                    vp = ps_acc.tile([128, 512], f32, tag="acc")
                    for pg in range(NPG):
                        nc.tensor.matmul(
                            vp[:],
                            vdst[pg][:, 128 * sb : 128 * (sb + 1)],
                            wvu[:, pg, 512 * et : 512 * (et + 1)],
                            start=(pg == 0),
                            stop=(pg == NPG - 1),
                        )
                    ob = obp.tile([128, 512], bf16, tag="ob")
                    nc.any.tensor_copy(ob[:], vp[:])
                    nc.gpsimd.dma_start(out_d[sb, :, 512 * et : 512 * (et + 1)], ob[:])

    nc.compile()
    return nc


def _get_nc():
    if "nc" not in _CACHE:
        _CACHE["nc"] = _build()
    return _CACHE["nc"]


def _prep_core_inputs(X, WQ, WK, WV_down, WV_up, core):
    """Build the per-core input map (host-side shard + layout + bf16 cast)."""
    b, g = core // 2, core % 2

    # X[b]^T chunks, partition-major: xt[p, ch, t] = X[b][t, 128*ch + p]
    xt = np.ascontiguousarray(
        X[b].T.reshape(8, 128, S).transpose(1, 0, 2).astype(BF16)
    )

    def pack_ekd(W, scale):
        # -> [128, NPG, 8, 128]: per pair-group, per e-chunk, [128e, 2x64d]
        out = np.empty((128, NPG, 8, 128), np.float32)
        for pg in range(NPG):
            h0 = 8 * g + 2 * pg
            pair = np.concatenate([W[h0], W[h0 + 1]], axis=1) * scale  # [E, 128]
            out[:, pg] = pair.reshape(8, 128, 128).transpose(1, 0, 2)
        return np.ascontiguousarray(out.astype(BF16))

    wq = pack_ekd(WQ, 1.0 / np.sqrt(np.float32(D)))
    wk = pack_ekd(WK, 1.0)
    wvd = pack_ekd(WV_down, 1.0)

    wvu = np.empty((128, NPG, E), np.float32)
    for pg in range(NPG):
        h0 = 8 * g + 2 * pg
        wvu[:, pg] = np.concatenate([WV_up[h0], WV_up[h0 + 1]], axis=0) / np.float32(N)
    wvu = np.ascontiguousarray(wvu.astype(BF16))

    iden = np.eye(128, dtype=np.float32).astype(BF16)
    trilm = np.tril(np.ones((128, 128), np.float32), -1)  # 1 where t<s (strict lower)
    sel2 = np.zeros((128, 2), np.float32)
    sel2[:64, 0] = 1.0
    sel2[64:, 1] = 1.0
    sel2 = sel2.astype(BF16)
    id2 = np.eye(2, dtype=np.float32)

    return {
        "xt": xt,
        "wq": wq,
        "wk": wk,
        "wvd": wvd,
        "wvu": wvu,
        "iden": iden,
        "trilm": trilm,
        "sel2": sel2,
        "id2": id2,
    }


def _install_trace_shim():
    """Provide antenv.axon_hooks (missing in this image) so trace=True can
    drive NTFF profiling via ctypes into libaxon_pjrt.so."""
    import contextlib
    import ctypes
    import types

    try:
        from antenv import axon_hooks  # noqa: F401

        return
    except ImportError:
        pass

    so_path = "/opt/axon/libaxon_pjrt.so"
    lib = ctypes.CDLL(so_path)
    if not hasattr(lib, "axon_start_nrt_profile"):
        hook = None
    else:
        lib.axon_start_nrt_profile.argtypes = [
            ctypes.POINTER(ctypes.c_int64),
            ctypes.c_size_t,
        ]
        lib.axon_start_nrt_profile.restype = ctypes.c_int64
        lib.axon_stop_nrt_profile.argtypes = [ctypes.c_char_p]
        lib.axon_stop_nrt_profile.restype = ctypes.c_int64

        @contextlib.contextmanager
        def hook(output_dir, device_ids):
            import jax

            jax.devices()
            if device_ids:
                ids = (ctypes.c_int64 * len(device_ids))(*device_ids)
                rc = lib.axon_start_nrt_profile(ids, len(device_ids))
            else:
                rc = lib.axon_start_nrt_profile(None, 0)
            if rc != 0:
                raise RuntimeError(f"axon_start_nrt_profile rc={rc}")
            try:
                yield
            finally:
                n = lib.axon_stop_nrt_profile(str(output_dir).encode())
                print(f"profile: {n} file(s) written to {output_dir}")

    mod = types.ModuleType("antenv.axon_hooks")
    mod.get_axon_ntff_profile_hook = lambda: hook
    mod.set_axon_ntff_profile_hook = lambda h: None
    sys.modules["antenv.axon_hooks"] = mod

    # artifact upload needs a bucket; degrade to local-only
    from concourse import bass_utils

    bass_utils.upload_artifacts = lambda tmpdir: str(tmpdir)


def kernel(inputs, WQ, WK, WV_down, WV_up, _trace=False):
    if _trace:
        _install_trace_shim()
    X = np.asarray(inputs, np.float32)
    WQ = np.asarray(WQ, np.float32)
    WK = np.asarray(WK, np.float32)
    WV_down = np.asarray(WV_down, np.float32)
    WV_up = np.asarray(WV_up, np.float32)

    nc = _get_nc()
    in_maps = [_prep_core_inputs(X, WQ, WK, WV_down, WV_up, c) for c in range(NCORES)]

    from concourse.bass_utils import run_bass_kernel_spmd

    res = run_bass_kernel_spmd(nc, in_maps, core_ids=list(range(NCORES)), trace=_trace)
    _CACHE["last_results"] = res

    out = np.empty((B, S, E), np.float32)
    for b in range(B):
        p0 = res.results[2 * b]["out"].astype(np.float32).reshape(S, E)
        p1 = res.results[2 * b + 1]["out"].astype(np.float32).reshape(S, E)
        out[b] = X[b] + p0 + p1
    return out


# revision 20
# speedup vs baseline: 1.0363x; 1.0363x over previous
"""Trainium2 Bass kernel for nn_MultiHeadSelfAttention_72748156059791.

Reference computation (B=4, S=1024, E=1024, N=16 heads, D=64):
    Q = X @ WQ[n];  K = X @ WK[n]
    scores = (K @ Q^T) / 8, upper-triangular kept, rest -> -inf
    attn = softmax(scores, axis=-1)
    v_down = diag(attn) * (X @ WV_down[n])     # only the DIAGONAL of attn is used!
    V = v_down @ WV_up[n]
    out = X + mean_n(V)

Key algorithmic facts exploited:
  * attn enters only through its diagonal -> no attn @ V matmul; we only need
    per-row softmax stats: Z[s] = sum_{t>=s} exp(scores[s,t]) and exp(scores[s,s]).
  * mean over heads is a linear op -> folded into WV_up (pre-scaled by 1/16)
    and the per-head V contributions are summed by PSUM accumulation using
    d-stacking: [vd_a | vd_b] @ [Wup_a ; Wup_b] = V_a + V_b.

Sharding: 8 cores = 4 batches x 2 head-groups (8 heads each). Each core
returns a [1024,1024] bf16 partial delta (sum of its 8 heads' V, already
/16); host adds the two partials per batch to the input.

All matmuls run in bf16 (fp32 PSUM accumulation); validated numerics:
scale-relative absmax error ~7e-5 vs the fp32 reference.
"""

import sys

import numpy as np
import ml_dtypes

if "/opt/trn_rl_repo" not in sys.path:
    sys.path.insert(0, "/opt/trn_rl_repo")

BF16 = ml_dtypes.bfloat16

B, S, E = 4, 1024, 1024
N, D = 16, 64
NCORES = 8
NPG = 4  # head pair-groups per core (2 heads each -> 8 heads/core)

_CACHE = {}


def _build():
    import concourse.bacc as bacc
    import concourse.tile as tile
    import concourse.mybir as mybir

    f32 = mybir.dt.float32
    bf16 = mybir.dt.bfloat16
    EXP = mybir.ActivationFunctionType.Exp

    nc = bacc.Bacc("TRN2", target_bir_lowering=False)

    xt_d = nc.dram_tensor("xt", [128, 8, 1024], bf16, kind="ExternalInput")
    wq_d = nc.dram_tensor("wq", [128, NPG, 8, 128], bf16, kind="ExternalInput")
    wk_d = nc.dram_tensor("wk", [128, NPG, 8, 128], bf16, kind="ExternalInput")
    wvd_d = nc.dram_tensor("wvd", [128, NPG, 8, 128], bf16, kind="ExternalInput")
    wvu_d = nc.dram_tensor("wvu", [128, NPG, 1024], bf16, kind="ExternalInput")
    iden_d = nc.dram_tensor("iden", [128, 128], bf16, kind="ExternalInput")
    trilm_d = nc.dram_tensor("trilm", [128, 128], f32, kind="ExternalInput")
    sel2_d = nc.dram_tensor("sel2", [128, 2], bf16, kind="ExternalInput")
    id2_d = nc.dram_tensor("id2", [2, 2], f32, kind="ExternalInput")
    out_d = nc.dram_tensor("out", [8, 128, 1024], bf16, kind="ExternalOutput")

    with tile.TileContext(nc) as tc:
        with (
            tc.tile_pool(name="cst", bufs=1) as cst,
            tc.tile_pool(name="wx", bufs=1) as wx,
            tc.tile_pool(name="proj", bufs=2) as proj,
            tc.tile_pool(name="vd4", bufs=4) as vd4,
            tc.tile_pool(name="st", bufs=2) as stp,
            tc.tile_pool(name="scr", bufs=1) as scrp,
            tc.tile_pool(name="ob", bufs=4) as obp,
            tc.tile_pool(name="ps_strip", bufs=2, space="PSUM") as ps_strip,
            tc.tile_pool(name="ps_proj", bufs=2, space="PSUM") as ps_proj,
            tc.tile_pool(name="ps_acc", bufs=2, space="PSUM") as ps_acc,
        ):
            # ---- constants & weights (loaded once) ----
            iden = cst.tile([128, 128], bf16, tag="iden")
            trilm = cst.tile([128, 128], f32, tag="trilm")
            sel2 = cst.tile([128, 2], bf16, tag="sel2")
            id2 = cst.tile([2, 2], f32, tag="id2")
            nc.sync.dma_start(iden[:], iden_d[:])
            nc.sync.dma_start(trilm[:], trilm_d[:])
            nc.sync.dma_start(sel2[:], sel2_d[:])
            nc.sync.dma_start(id2[:], id2_d[:])

            xt = wx.tile([128, 8, 1024], bf16, tag="xt")
            wq = wx.tile([128, NPG, 8, 128], bf16, tag="wq")
            wk = wx.tile([128, NPG, 8, 128], bf16, tag="wk")
            wvd = wx.tile([128, NPG, 8, 128], bf16, tag="wvd")
            wvu = wx.tile([128, NPG, 1024], bf16, tag="wvu")
            # pg0 weights + x chunks first so projections start ASAP
            nc.sync.dma_start(wq[:, 0, :, :], wq_d[:, 0, :, :])
            for ch in range(8):
                nc.sync.dma_start(xt[:, ch, :], xt_d[:, ch, :])
            nc.sync.dma_start(wk[:, 0, :, :], wk_d[:, 0, :, :])
            nc.sync.dma_start(wvd[:, 0, :, :], wvd_d[:, 0, :, :])
            for pg in range(1, NPG):
                for t, d in ((wq, wq_d), (wk, wk_d), (wvd, wvd_d)):
                    nc.sync.dma_start(t[:, pg, :, :], d[:, pg, :, :])
            for pg in range(NPG):
                nc.sync.dma_start(wvu[:, pg, :], wvu_d[:, pg, :])

            vdst = [vd4.tile([128, 1024], bf16, tag="vdst", name=f"vdst{_pg}") for _pg in range(NPG)]

            # per-pg projection targets; groups are emitted lazily so that the
            # next pg's projection matmuls interleave with (ACT-bound) scores
            # strips of the current pg, keeping TensorE dense (HAM warm).
            projs = {}

            def alloc_proj(pg):
                q_sb = proj.tile([128, 1024], bf16, tag="q", name=f"q{pg}")
                k_sb = proj.tile([128, 1024], bf16, tag="k", name=f"k{pg}")
                vd_sb = proj.tile([128, 1024], bf16, tag="vd", name=f"vd{pg}")
                projs[pg] = (q_sb, k_sb, vd_sb)

            def emit_proj_group(pg, which, sthalf):
                wsb = (wq, wk, wvd)[which]
                dst = projs[pg][which]
                ps = ps_proj.tile(
                    [128, 512], f32, tag="pp", name=f"pp_{pg}_{which}_{sthalf}"
                )
                cols = slice(512 * sthalf, 512 * (sthalf + 1))
                for ch in range(8):
                    nc.tensor.matmul(
                        ps[:],
                        wsb[:, pg, ch, :],
                        xt[:, ch, cols],
                        start=(ch == 0),
                        stop=(ch == 7),
                    )
                nc.vector.tensor_copy(dst[:, cols], ps[:])

            alloc_proj(0)
            for which in range(3):
                for sthalf in range(2):
                    emit_proj_group(0, which, sthalf)

            def emit_bcast_vdst(pg, sthalf, reps, vd_sb):
                bc = ps_acc.tile(
                    [128, 512], f32, tag="acc", name=f"bc{pg}_{sthalf}"
                )
                for il in range(4):
                    i = 4 * sthalf + il
                    for h2 in range(2):
                        nc.tensor.matmul(
                            bc[64 * h2 : 64 * (h2 + 1), 128 * il : 128 * (il + 1)],
                            reps[h2][:, i : i + 1].broadcast_to((128, 64)),
                            iden[:],
                            start=True,
                            stop=True,
                            tile_position=(0, 64 * h2),
                        )
                cols = slice(512 * sthalf, 512 * (sthalf + 1))
                nc.vector.tensor_mul(vdst[pg][:, cols], vd_sb[:, cols], bc)

            for pg in range(NPG):
                q_sb, k_sb, vd_sb = projs[pg]
                pending = []
                if pg + 1 < NPG:
                    alloc_proj(pg + 1)
                    pending = [
                        (lambda a=pg + 1, w=w, s=s: emit_proj_group(a, w, s))
                        for w in range(3)
                        for s in range(2)
                    ]

                # ---- scores row-strips + unmasked exp (accumulates Za incl.
                # sub-diagonal junk of the diag block; corrected below) ----
                zas = [stp.tile([128, 8], f32, tag=f"za{h2}", name=f"za{h2}") for h2 in range(2)]
                scr = [
                    scrp.tile([128, 8, 1024], f32, tag=f"scr{h2}", name=f"scr{h2}")
                    for h2 in range(2)
                ]
                for i in range(8):
                    c0 = 128 * i
                    strips = []
                    for h2 in range(2):
                        pss = ps_strip.tile([128, 1024], f32, tag="strip")
                        strips.append(pss)
                        hp = slice(64 * h2, 64 * (h2 + 1))
                        segs = [(c0, 512), (512, 1024)] if c0 < 512 else [(c0, 1024)]
                        for lo, hi in segs:
                            nc.tensor.matmul(
                                pss[:, lo:hi],
                                k_sb[hp, c0 : c0 + 128],
                                q_sb[hp, lo:hi],
                                start=True,
                                stop=True,
                                tile_position=(64 * h2, 0),
                            )
                    for h2 in range(2):
                        nc.scalar.activation(
                            scr[h2][:, i, 0 : 1024 - c0],
                            strips[h2][:, c0:1024],
                            EXP,
                            accum_out=zas[h2][:, i : i + 1],
                        )
                    if pending:
                        pending.pop(0)()
                while pending:
                    pending.pop(0)()

                # ---- diagonal of scores: d[s] = K[s].Q[s], via K^T*Q^T + selector matmul ----
                p_sb = proj.tile([128, 1024], bf16, tag="p")
                nc.vector.tensor_mul(p_sb[:], q_sb[:], k_sb[:])
                edt = stp.tile([2, 1024], f32, tag="edt")
                dt_ps = ps_strip.tile([2, 1024], f32, tag="strip")
                for sthalf in range(2):
                    cols = slice(512 * sthalf, 512 * (sthalf + 1))
                    nc.tensor.matmul(
                        dt_ps[:, cols], sel2[:], p_sb[:, cols], start=True, stop=True
                    )
                nc.scalar.activation(edt[:], dt_ps[:], EXP)
                # transpose exp(diag) back to [s-partition] layout, then evacuate
                # to SBUF immediately to free the PSUM slot
                ed_ps = ps_proj.tile([128, 2, 8], f32, tag="pp", name=f"edps{pg}")
                for i in range(8):
                    nc.tensor.transpose(
                        ed_ps[:, :, i], edt[:, 128 * i : 128 * (i + 1)], id2[:]
                    )
                ed_sb = stp.tile([128, 2, 8], f32, tag="ed_sb")
                nc.vector.tensor_copy(ed_sb[:], ed_ps[:])

                # ---- attn diag: Z = Za - (sub-diagonal junk sums), batched ----
                reps = []
                for h2 in range(2):
                    tmpm = stp.tile([128, 8, 128], f32, tag=f"tmpm{h2}")
                    nc.vector.tensor_mul(
                        tmpm[:],
                        scr[h2][:, :, 0:128],
                        trilm[:].unsqueeze(1).broadcast_to((128, 8, 128)),
                    )
                    zc = stp.tile([128, 8], f32, tag=f"zc{h2}")
                    nc.vector.tensor_reduce(
                        zc[:], tmpm[:], axis=mybir.AxisListType.X, op=mybir.AluOpType.add
                    )
                    z = stp.tile([128, 8], f32, tag=f"z{h2}")
                    nc.vector.tensor_sub(z[:], zas[h2][:], zc[:])
                    r = stp.tile([128, 8], f32, tag=f"r{h2}")
                    nc.vector.reciprocal(r[:], z[:])
                    at = stp.tile([128, 8], bf16, tag=f"at{h2}")
                    nc.vector.tensor_mul(at[:], r[:], ed_sb[:, h2, :])
                    reps.append(at)
                for sthalf in range(2):
                    emit_bcast_vdst(pg, sthalf, reps, vd_sb)

            # ---- V up-projection: accumulate all 8 heads (4 pg) in PSUM ----
            for sb in range(8):
                for et in range(2):
                    vp = ps_acc.tile([128, 512], f32, tag="acc")
                    for pg in range(NPG):
                        nc.tensor.matmul(
                            vp[:],
                            vdst[pg][:, 128 * sb : 128 * (sb + 1)],
                            wvu[:, pg, 512 * et : 512 * (et + 1)],
                            start=(pg == 0),
                            stop=(pg == NPG - 1),
                        )
                    ob = obp.tile([128, 512], bf16, tag="ob")
                    nc.any.tensor_copy(ob[:], vp[:])
                    nc.sync.dma_start(out_d[sb, :, 512 * et : 512 * (et + 1)], ob)

    nc.compile()
    return nc


def _get_nc():
    if "nc" not in _CACHE:
        _CACHE["nc"] = _build()
    return _CACHE["nc"]


def _prep_core_inputs(X, WQ, WK, WV_down, WV_up, core):
    """Build the per-core input map (host-side shard + layout + bf16 cast)."""
    b, g = core // 2, core % 2

    # X[b]^T chunks, partition-major: xt[p, ch, t] = X[b][t, 128*ch + p]
    xt = np.ascontiguousarray(
        X[b].T.reshape(8, 128, S).transpose(1, 0, 2).astype(BF16)
    )

    def pack_ekd(W, scale):
        # -> [128, NPG, 8, 128]: per pair-group, per e-chunk, [128e, 2x64d]
        out = np.empty((128, NPG, 8, 128), np.float32)
        for pg in range(NPG):
            h0 = 8 * g + 2 * pg
            pair = np.concatenate([W[h0], W[h0 + 1]], axis=1) * scale  # [E, 128]
            out[:, pg] = pair.reshape(8, 128, 128).transpose(1, 0, 2)
        return np.ascontiguousarray(out.astype(BF16))

    wq = pack_ekd(WQ, 1.0 / np.sqrt(np.float32(D)))
    wk = pack_ekd(WK, 1.0)
    wvd = pack_ekd(WV_down, 1.0)

    wvu = np.empty((128, NPG, E), np.float32)
    for pg in range(NPG):
        h0 = 8 * g + 2 * pg
        wvu[:, pg] = np.concatenate([WV_up[h0], WV_up[h0 + 1]], axis=0) / np.float32(N)
    wvu = np.ascontiguousarray(wvu.astype(BF16))

    iden = np.eye(128, dtype=np.float32).astype(BF16)
    trilm = np.tril(np.ones((128, 128), np.float32), -1)  # 1 where t<s (strict lower)
    sel2 = np.zeros((128, 2), np.float32)
    sel2[:64, 0] = 1.0
    sel2[64:, 1] = 1.0
    sel2 = sel2.astype(BF16)
    id2 = np.eye(2, dtype=np.float32)

    return {
        "xt": xt,
        "wq": wq,
        "wk": wk,
        "wvd": wvd,
        "wvu": wvu,
        "iden": iden,
        "trilm": trilm,
        "sel2": sel2,
        "id2": id2,
    }


def _install_trace_shim():
    """Provide antenv.axon_hooks (missing in this image) so trace=True can
    drive NTFF profiling via ctypes into libaxon_pjrt.so."""
    import contextlib
    import ctypes
    import types

    try:
        from antenv import axon_hooks  # noqa: F401

        return
    except ImportError:
        pass

    so_path = "/opt/axon/libaxon_pjrt.so"
    lib = ctypes.CDLL(so_path)
    if not hasattr(lib, "axon_start_nrt_profile"):
        hook = None
    else:
        lib.axon_start_nrt_profile.argtypes = [
            ctypes.POINTER(ctypes.c_int64),
            ctypes.c_size_t,
        ]
        lib.axon_start_nrt_profile.restype = ctypes.c_int64
        lib.axon_stop_nrt_profile.argtypes = [ctypes.c_char_p]
        lib.axon_stop_nrt_profile.restype = ctypes.c_int64

        @contextlib.contextmanager
        def hook(output_dir, device_ids):
            import jax

            jax.devices()
            if device_ids:
                ids = (ctypes.c_int64 * len(device_ids))(*device_ids)
                rc = lib.axon_start_nrt_profile(ids, len(device_ids))
            else:
                rc = lib.axon_start_nrt_profile(None, 0)
            if rc != 0:
                raise RuntimeError(f"axon_start_nrt_profile rc={rc}")
            try:
                yield
            finally:
                n = lib.axon_stop_nrt_profile(str(output_dir).encode())
                print(f"profile: {n} file(s) written to {output_dir}")

    mod = types.ModuleType("antenv.axon_hooks")
    mod.get_axon_ntff_profile_hook = lambda: hook
    mod.set_axon_ntff_profile_hook = lambda h: None
    sys.modules["antenv.axon_hooks"] = mod

    # artifact upload needs a bucket; degrade to local-only
    from concourse import bass_utils

    bass_utils.upload_artifacts = lambda tmpdir: str(tmpdir)


def kernel(inputs, WQ, WK, WV_down, WV_up, _trace=False):
    if _trace:
        _install_trace_shim()
    X = np.asarray(inputs, np.float32)
    WQ = np.asarray(WQ, np.float32)
    WK = np.asarray(WK, np.float32)
    WV_down = np.asarray(WV_down, np.float32)
    WV_up = np.asarray(WV_up, np.float32)

    nc = _get_nc()
    in_maps = [_prep_core_inputs(X, WQ, WK, WV_down, WV_up, c) for c in range(NCORES)]

    from concourse.bass_utils import run_bass_kernel_spmd

    res = run_bass_kernel_spmd(nc, in_maps, core_ids=list(range(NCORES)), trace=_trace)
    _CACHE["last_results"] = res

    out = np.empty((B, S, E), np.float32)
    for b in range(B):
        p0 = res.results[2 * b]["out"].astype(np.float32).reshape(S, E)
        p1 = res.results[2 * b + 1]["out"].astype(np.float32).reshape(S, E)
        out[b] = X[b] + p0 + p1
    return out


# revision 21
# speedup vs baseline: 1.0487x; 1.0120x over previous
"""Trainium2 Bass kernel for nn_MultiHeadSelfAttention_72748156059791.

Reference computation (B=4, S=1024, E=1024, N=16 heads, D=64):
    Q = X @ WQ[n];  K = X @ WK[n]
    scores = (K @ Q^T) / 8, upper-triangular kept, rest -> -inf
    attn = softmax(scores, axis=-1)
    v_down = diag(attn) * (X @ WV_down[n])     # only the DIAGONAL of attn is used!
    V = v_down @ WV_up[n]
    out = X + mean_n(V)

Key algorithmic facts exploited:
  * attn enters only through its diagonal -> no attn @ V matmul; we only need
    per-row softmax stats: Z[s] = sum_{t>=s} exp(scores[s,t]) and exp(scores[s,s]).
  * mean over heads is a linear op -> folded into WV_up (pre-scaled by 1/16)
    and the per-head V contributions are summed by PSUM accumulation using
    d-stacking: [vd_a | vd_b] @ [Wup_a ; Wup_b] = V_a + V_b.

Sharding: 8 cores = 4 batches x 2 head-groups (8 heads each). Each core
returns a [1024,1024] bf16 partial delta (sum of its 8 heads' V, already
/16); host adds the two partials per batch to the input.

All matmuls run in bf16 (fp32 PSUM accumulation); validated numerics:
scale-relative absmax error ~7e-5 vs the fp32 reference.
"""

import sys

import numpy as np
import ml_dtypes

if "/opt/trn_rl_repo" not in sys.path:
    sys.path.insert(0, "/opt/trn_rl_repo")

BF16 = ml_dtypes.bfloat16

B, S, E = 4, 1024, 1024
N, D = 16, 64
NCORES = 8
NPG = 4  # head pair-groups per core (2 heads each -> 8 heads/core)

_CACHE = {}


def _build():
    import concourse.bacc as bacc
    import concourse.tile as tile
    import concourse.mybir as mybir

    f32 = mybir.dt.float32
    bf16 = mybir.dt.bfloat16
    EXP = mybir.ActivationFunctionType.Exp

    nc = bacc.Bacc("TRN2", target_bir_lowering=False)

    xt_d = nc.dram_tensor("xt", [128, 8, 1024], bf16, kind="ExternalInput")
    wq_d = nc.dram_tensor("wq", [128, NPG, 8, 128], bf16, kind="ExternalInput")
    wk_d = nc.dram_tensor("wk", [128, NPG, 8, 128], bf16, kind="ExternalInput")
    wvd_d = nc.dram_tensor("wvd", [128, NPG, 8, 128], bf16, kind="ExternalInput")
    wvu_d = nc.dram_tensor("wvu", [128, NPG, 1024], bf16, kind="ExternalInput")
    iden_d = nc.dram_tensor("iden", [128, 128], bf16, kind="ExternalInput")
    trilm_d = nc.dram_tensor("trilm", [128, 128], f32, kind="ExternalInput")
    sel2_d = nc.dram_tensor("sel2", [128, 2], bf16, kind="ExternalInput")
    id2_d = nc.dram_tensor("id2", [2, 2], f32, kind="ExternalInput")
    out_d = nc.dram_tensor("out", [8, 128, 1024], bf16, kind="ExternalOutput")

    with tile.TileContext(nc) as tc:
        with (
            tc.tile_pool(name="cst", bufs=1) as cst,
            tc.tile_pool(name="wx", bufs=1) as wx,
            tc.tile_pool(name="proj", bufs=2) as proj,
            tc.tile_pool(name="vd4", bufs=4) as vd4,
            tc.tile_pool(name="st", bufs=2) as stp,
            tc.tile_pool(name="scr", bufs=1) as scrp,
            tc.tile_pool(name="ob", bufs=4) as obp,
            tc.tile_pool(name="ps_strip", bufs=2, space="PSUM") as ps_strip,
            tc.tile_pool(name="ps_proj", bufs=2, space="PSUM") as ps_proj,
            tc.tile_pool(name="ps_acc", bufs=2, space="PSUM") as ps_acc,
        ):
            # ---- constants & weights (loaded once) ----
            iden = cst.tile([128, 128], bf16, tag="iden")
            trilm = cst.tile([128, 128], f32, tag="trilm")
            sel2 = cst.tile([128, 2], bf16, tag="sel2")
            id2 = cst.tile([2, 2], f32, tag="id2")
            nc.sync.dma_start(iden[:], iden_d[:])
            nc.sync.dma_start(trilm[:], trilm_d[:])
            nc.sync.dma_start(sel2[:], sel2_d[:])
            nc.sync.dma_start(id2[:], id2_d[:])

            xt = wx.tile([128, 8, 1024], bf16, tag="xt")
            wq = wx.tile([128, NPG, 8, 128], bf16, tag="wq")
            wk = wx.tile([128, NPG, 8, 128], bf16, tag="wk")
            wvd = wx.tile([128, NPG, 8, 128], bf16, tag="wvd")
            wvu = wx.tile([128, NPG, 1024], bf16, tag="wvu")
            # pg0 weights + x chunks first so projections start ASAP
            nc.sync.dma_start(wq[:, 0, :, :], wq_d[:, 0, :, :])
            for ch in range(8):
                nc.sync.dma_start(xt[:, ch, :], xt_d[:, ch, :])
            nc.sync.dma_start(wk[:, 0, :, :], wk_d[:, 0, :, :])
            nc.sync.dma_start(wvd[:, 0, :, :], wvd_d[:, 0, :, :])
            for pg in range(1, NPG):
                for t, d in ((wq, wq_d), (wk, wk_d), (wvd, wvd_d)):
                    nc.sync.dma_start(t[:, pg, :, :], d[:, pg, :, :])
            for pg in range(NPG):
                nc.sync.dma_start(wvu[:, pg, :], wvu_d[:, pg, :])

            vdst = [vd4.tile([128, 1024], bf16, tag="vdst", name=f"vdst{_pg}") for _pg in range(NPG)]

            # per-pg projection targets; groups are emitted lazily so that the
            # next pg's projection matmuls interleave with (ACT-bound) scores
            # strips of the current pg, keeping TensorE dense (HAM warm).
            projs = {}

            def alloc_proj(pg):
                q_sb = proj.tile([128, 1024], bf16, tag="q", name=f"q{pg}")
                k_sb = proj.tile([128, 1024], bf16, tag="k", name=f"k{pg}")
                vd_sb = proj.tile([128, 1024], bf16, tag="vd", name=f"vd{pg}")
                projs[pg] = (q_sb, k_sb, vd_sb)

            def emit_proj_group(pg, which, sthalf):
                wsb = (wq, wk, wvd)[which]
                dst = projs[pg][which]
                ps = ps_proj.tile(
                    [128, 512], f32, tag="pp", name=f"pp_{pg}_{which}_{sthalf}"
                )
                cols = slice(512 * sthalf, 512 * (sthalf + 1))
                for ch in range(8):
                    nc.tensor.matmul(
                        ps[:],
                        wsb[:, pg, ch, :],
                        xt[:, ch, cols],
                        start=(ch == 0),
                        stop=(ch == 7),
                    )
                nc.vector.tensor_copy(dst[:, cols], ps[:])

            alloc_proj(0)
            for which in range(3):
                for sthalf in range(2):
                    emit_proj_group(0, which, sthalf)

            def emit_bcast_vdst(pg, sthalf, reps, vd_sb):
                bc = ps_acc.tile(
                    [128, 512], f32, tag="acc", name=f"bc{pg}_{sthalf}"
                )
                for il in range(4):
                    i = 4 * sthalf + il
                    for h2 in range(2):
                        nc.tensor.matmul(
                            bc[64 * h2 : 64 * (h2 + 1), 128 * il : 128 * (il + 1)],
                            reps[h2][:, i : i + 1].broadcast_to((128, 64)),
                            iden[:],
                            start=True,
                            stop=True,
                            tile_position=(0, 64 * h2),
                        )
                cols = slice(512 * sthalf, 512 * (sthalf + 1))
                nc.vector.tensor_mul(vdst[pg][:, cols], vd_sb[:, cols], bc)

            for pg in range(NPG):
                q_sb, k_sb, vd_sb = projs[pg]
                pending = []
                if pg + 1 < NPG:
                    alloc_proj(pg + 1)
                    pending = [
                        (lambda a=pg + 1, w=w, s=s: emit_proj_group(a, w, s))
                        for w in range(3)
                        for s in range(2)
                    ]

                # ---- scores row-strips + unmasked exp (accumulates Za incl.
                # sub-diagonal junk of the diag block; corrected below) ----
                zas = [stp.tile([128, 8], f32, tag=f"za{h2}", name=f"za{h2}") for h2 in range(2)]
                scr = [
                    scrp.tile([128, 8, 1024], f32, tag=f"scr{h2}", name=f"scr{h2}")
                    for h2 in range(2)
                ]
                for i in range(8):
                    c0 = 128 * i
                    strips = []
                    for h2 in range(2):
                        pss = ps_strip.tile([128, 1024], f32, tag="strip")
                        strips.append(pss)
                        hp = slice(64 * h2, 64 * (h2 + 1))
                        segs = [(c0, 512), (512, 1024)] if c0 < 512 else [(c0, 1024)]
                        for lo, hi in segs:
                            nc.tensor.matmul(
                                pss[:, lo:hi],
                                k_sb[hp, c0 : c0 + 128],
                                q_sb[hp, lo:hi],
                                start=True,
                                stop=True,
                                tile_position=(64 * h2, 0),
                            )
                    for h2 in range(2):
                        nc.scalar.activation(
                            scr[h2][:, i, 0 : 1024 - c0],
                            strips[h2][:, c0:1024],
                            EXP,
                            accum_out=zas[h2][:, i : i + 1],
                        )
                    if pending:
                        pending.pop(0)()
                while pending:
                    pending.pop(0)()

                # ---- diagonal of scores: d[s] = K[s].Q[s], via K^T*Q^T + selector matmul ----
                p_sb = proj.tile([128, 1024], bf16, tag="p")
                nc.vector.tensor_mul(p_sb[:], q_sb[:], k_sb[:])
                edt = stp.tile([2, 1024], f32, tag="edt")
                dt_ps = ps_strip.tile([2, 1024], f32, tag="strip")
                for sthalf in range(2):
                    cols = slice(512 * sthalf, 512 * (sthalf + 1))
                    nc.tensor.matmul(
                        dt_ps[:, cols], sel2[:], p_sb[:, cols], start=True, stop=True
                    )
                nc.scalar.activation(edt[:], dt_ps[:], EXP)
                # transpose exp(diag) back to [s-partition] layout, then evacuate
                # to SBUF immediately to free the PSUM slot
                ed_ps = ps_proj.tile([128, 2, 8], f32, tag="pp", name=f"edps{pg}")
                for i in range(8):
                    nc.tensor.transpose(
                        ed_ps[:, :, i], edt[:, 128 * i : 128 * (i + 1)], id2[:]
                    )
                ed_sb = stp.tile([128, 2, 8], f32, tag="ed_sb")
                nc.vector.tensor_copy(ed_sb[:], ed_ps[:])

                # ---- attn diag: Z = Za - (sub-diagonal junk sums), batched ----
                reps = []
                for h2 in range(2):
                    tmpm = stp.tile([128, 8, 128], f32, tag=f"tmpm{h2}")
                    nc.vector.tensor_mul(
                        tmpm[:],
                        scr[h2][:, :, 0:128],
                        trilm[:].unsqueeze(1).broadcast_to((128, 8, 128)),
                    )
                    zc = stp.tile([128, 8], f32, tag=f"zc{h2}")
                    nc.vector.tensor_reduce(
                        zc[:], tmpm[:], axis=mybir.AxisListType.X, op=mybir.AluOpType.add
                    )
                    z = stp.tile([128, 8], f32, tag=f"z{h2}")
                    nc.vector.tensor_sub(z[:], zas[h2][:], zc[:])
                    r = stp.tile([128, 8], f32, tag=f"r{h2}")
                    nc.vector.reciprocal(r[:], z[:])
                    at = stp.tile([128, 8], bf16, tag=f"at{h2}")
                    nc.vector.tensor_mul(at[:], r[:], ed_sb[:, h2, :])
                    reps.append(at)
                for sthalf in range(2):
                    emit_bcast_vdst(pg, sthalf, reps, vd_sb)

            # ---- V up-projection: accumulate all 8 heads (4 pg) in PSUM ----
            for sb in range(8):
                for et in range(2):
                    vp = ps_acc.tile([128, 512], f32, tag="acc")
                    for pg in range(NPG):
                        nc.tensor.matmul(
                            vp[:],
                            vdst[pg][:, 128 * sb : 128 * (sb + 1)],
                            wvu[:, pg, 512 * et : 512 * (et + 1)],
                            start=(pg == 0),
                            stop=(pg == NPG - 1),
                        )
                    ob = obp.tile([128, 512], bf16, tag="ob")
                    nc.any.tensor_copy(ob[:], vp[:])
                    nc.gpsimd.dma_start(out_d[sb, :, 512 * et : 512 * (et + 1)], ob[:])

    nc.compile()
    return nc


def _get_nc():
    if "nc" not in _CACHE:
        _CACHE["nc"] = _build()
    return _CACHE["nc"]


def _prep_core_inputs(X, WQ, WK, WV_down, WV_up, core):
    """Build the per-core input map (host-side shard + layout + bf16 cast)."""
    b, g = core // 2, core % 2

    # X[b]^T chunks, partition-major: xt[p, ch, t] = X[b][t, 128*ch + p]
    xt = np.ascontiguousarray(
        X[b].T.reshape(8, 128, S).transpose(1, 0, 2).astype(BF16)
    )

    def pack_ekd(W, scale):
        # -> [128, NPG, 8, 128]: per pair-group, per e-chunk, [128e, 2x64d]
        out = np.empty((128, NPG, 8, 128), np.float32)
        for pg in range(NPG):
            h0 = 8 * g + 2 * pg
            pair = np.concatenate([W[h0], W[h0 + 1]], axis=1) * scale  # [E, 128]
            out[:, pg] = pair.reshape(8, 128, 128).transpose(1, 0, 2)
        return np.ascontiguousarray(out.astype(BF16))

    wq = pack_ekd(WQ, 1.0 / np.sqrt(np.float32(D)))
    wk = pack_ekd(WK, 1.0)
    wvd = pack_ekd(WV_down, 1.0)

    wvu = np.empty((128, NPG, E), np.float32)
    for pg in range(NPG):
        h0 = 8 * g + 2 * pg
        wvu[:, pg] = np.concatenate([WV_up[h0], WV_up[h0 + 1]], axis=0) / np.float32(N)
    wvu = np.ascontiguousarray(wvu.astype(BF16))

    iden = np.eye(128, dtype=np.float32).astype(BF16)
    trilm = np.tril(np.ones((128, 128), np.float32), -1)  # 1 where t<s (strict lower)
    sel2 = np.zeros((128, 2), np.float32)
    sel2[:64, 0] = 1.0
    sel2[64:, 1] = 1.0
    sel2 = sel2.astype(BF16)
    id2 = np.eye(2, dtype=np.float32)

    return {
        "xt": xt,
        "wq": wq,
        "wk": wk,
        "wvd": wvd,
        "wvu": wvu,
        "iden": iden,
        "trilm": trilm,
        "sel2": sel2,
        "id2": id2,
    }


def _install_trace_shim():
    """Provide antenv.axon_hooks (missing in this image) so trace=True can
    drive NTFF profiling via ctypes into libaxon_pjrt.so."""
    import contextlib
    import ctypes
    import types

    try:
        from antenv import axon_hooks  # noqa: F401

        return
    except ImportError:
        pass

    so_path = "/opt/axon/libaxon_pjrt.so"
    lib = ctypes.CDLL(so_path)
    if not hasattr(lib, "axon_start_nrt_profile"):
        hook = None
    else:
        lib.axon_start_nrt_profile.argtypes = [
            ctypes.POINTER(ctypes.c_int64),
            ctypes.c_size_t,
        ]
        lib.axon_start_nrt_profile.restype = ctypes.c_int64
        lib.axon_stop_nrt_profile.argtypes = [ctypes.c_char_p]
        lib.axon_stop_nrt_profile.restype = ctypes.c_int64

        @contextlib.contextmanager
        def hook(output_dir, device_ids):
            import jax

            jax.devices()
            if device_ids:
                ids = (ctypes.c_int64 * len(device_ids))(*device_ids)
                rc = lib.axon_start_nrt_profile(ids, len(device_ids))
            else:
                rc = lib.axon_start_nrt_profile(None, 0)
            if rc != 0:
                raise RuntimeError(f"axon_start_nrt_profile rc={rc}")
            try:
                yield
            finally:
                n = lib.axon_stop_nrt_profile(str(output_dir).encode())
                print(f"profile: {n} file(s) written to {output_dir}")

    mod = types.ModuleType("antenv.axon_hooks")
    mod.get_axon_ntff_profile_hook = lambda: hook
    mod.set_axon_ntff_profile_hook = lambda h: None
    sys.modules["antenv.axon_hooks"] = mod

    # artifact upload needs a bucket; degrade to local-only
    from concourse import bass_utils

    bass_utils.upload_artifacts = lambda tmpdir: str(tmpdir)


def kernel(inputs, WQ, WK, WV_down, WV_up, _trace=False):
    if _trace:
        _install_trace_shim()
    X = np.asarray(inputs, np.float32)
    WQ = np.asarray(WQ, np.float32)
    WK = np.asarray(WK, np.float32)
    WV_down = np.asarray(WV_down, np.float32)
    WV_up = np.asarray(WV_up, np.float32)

    nc = _get_nc()
    in_maps = [_prep_core_inputs(X, WQ, WK, WV_down, WV_up, c) for c in range(NCORES)]

    from concourse.bass_utils import run_bass_kernel_spmd

    res = run_bass_kernel_spmd(nc, in_maps, core_ids=list(range(NCORES)), trace=_trace)
    _CACHE["last_results"] = res

    out = np.empty((B, S, E), np.float32)
    for b in range(B):
        p0 = res.results[2 * b]["out"].astype(np.float32).reshape(S, E)
        p1 = res.results[2 * b + 1]["out"].astype(np.float32).reshape(S, E)
        out[b] = X[b] + p0 + p1
    return out
